# revision 11
# baseline (speedup 1.0000x reference)
"""Trainium2 Bass kernel for y = enc_x @ weight.T + bias.

Shapes (hardcoded): enc_x [524288, 128] f32, weight [128, 128] f32,
bias [128] f32 -> y [524288, 128] f32.

Strategy: data-parallel over 8 NeuronCores (65536 rows each), bf16 on the
wire. The tolerance gate (rel err < 2e-2) leaves ample room for bf16 I/O:
quantizing x and W to bf16 and the output y to bf16 gives ~4e-3 max rel
error while halving HBM traffic (the problem is memory-bound).

The host uploads x pre-transposed per core (x^T [128, 65536] bf16, feature
dim on partitions), so the device needs no on-chip transposes at all: the
tensor engine computes y^T = (W^T)^T-stationary @ x^T directly with N=512
streaming matmuls (W^T [128i, 128o] is the stationary operand, loaded from
SBUF each MM; x^T streams). PSUM fp32 accumulation; the vector engine adds
bias (per-partition scalar) while evicting PSUM -> SBUF with a bf16 cast.
The host transposes y^T back and upcasts to f32.

Input DMAs ride the SP HWDGE ring (nc.sync), output DMAs the ACT ring
(nc.scalar) so a store blocked on compute never head-of-line-blocks the
next prefetch.
"""

import numpy as np

B, IN, OUT = 524288, 128, 128
N_CORES = 8
ROWS = B // N_CORES            # 65536 rows per core
CHUNK = 4096                   # batch columns per SBUF tile (1 MiB bf16 DMA)
N_CHUNKS = ROWS // CHUNK       # 16
GROUP = 512                    # matmul N / one PSUM bank of f32
GROUPS = CHUNK // GROUP        # 8

_CACHE: dict = {}


def _build():
    import concourse.bacc as bacc
    import concourse.mybir as mybir
    import concourse.tile as tile
    from concourse.bass import ts

    nc = bacc.Bacc(
        "TRN2",
        target_bir_lowering=False,
        debug=False,
        enable_asserts=False,
        num_devices=N_CORES,
    )

    f32 = mybir.dt.float32
    bf16 = mybir.dt.bfloat16
    xt_d = nc.dram_tensor("xt", [IN, ROWS], bf16, kind="ExternalInput").ap()
    wt_d = nc.dram_tensor("wt", [IN, OUT], bf16, kind="ExternalInput").ap()
    bc_d = nc.dram_tensor("bc", [OUT, 1], f32, kind="ExternalInput").ap()
    yt_d = nc.dram_tensor("yt", [OUT, ROWS], bf16, kind="ExternalOutput").ap()

    with tile.TileContext(nc) as tc:
        with (
            tc.tile_pool(name="consts", bufs=1) as cpool,
            tc.tile_pool(name="xin", bufs=4) as xpool,
            tc.tile_pool(name="yout", bufs=8) as ypool,
            tc.tile_pool(name="ps", bufs=3, space="PSUM") as pspool,
            tc.tile_pool(name="warm", bufs=1, space="PSUM") as wpool,
        ):
            # Consts ride the ACT ring so the SP ring's first DMA is X[0].
            wt_sb = cpool.tile([IN, OUT], bf16)
            nc.scalar.dma_start(wt_sb[:], wt_d)
            b_sb = cpool.tile([OUT, 1], f32)
            nc.scalar.dma_start(b_sb[:], bc_d)

            # The PE HAM throttles the clock to 1.2 GHz after an idle
            # activity window; this burst of scratch matmuls runs while the
            # first input chunk is still in flight (the PE is idle anyway)
            # so the first real matmuls start at the warm 2.4 GHz clock.
            scratch = wpool.tile([OUT, OUT], f32)
            for _ in range(10):
                nc.tensor.matmul(
                    scratch[:], wt_sb[:], wt_sb[:], start=True, stop=True
                )

            for c in range(N_CHUNKS):
                X = xpool.tile([IN, CHUNK], bf16, tag="X")
                nc.sync.dma_start(X[:], xt_d[:, c * CHUNK : (c + 1) * CHUNK])
                YT = ypool.tile([OUT, CHUNK], bf16, tag="YT")
                for g in range(GROUPS):
                    ps = pspool.tile([OUT, GROUP], f32, tag="ps")
                    nc.tensor.matmul(
                        ps[:],
                        wt_sb[:],
                        X[:, ts(g, GROUP)],
                        start=True,
                        stop=True,
                    )
                    # PSUM eviction (+bias, f32->bf16 cast) alternates between
                    # DVE and ACT so neither engine's throughput binds.
                    if g % 2 == 0:
                        nc.vector.tensor_scalar_add(
                            YT[:, ts(g, GROUP)], ps[:], b_sb[:]
                        )
                    else:
                        nc.scalar.add(YT[:, ts(g, GROUP)], ps[:], b_sb[:])
                nc.scalar.dma_start(yt_d[:, c * CHUNK : (c + 1) * CHUNK], YT[:])

    nc.compile()
    return nc


def _get_nc():
    if "nc" not in _CACHE:
        _CACHE["nc"] = _build()
    return _CACHE["nc"]


def prep_in_maps(enc_x: np.ndarray, weight: np.ndarray, bias: np.ndarray):
    import ml_dtypes

    bf16 = ml_dtypes.bfloat16
    wt = np.ascontiguousarray(weight.astype(np.float32).T.astype(bf16))  # [IN, OUT]
    bc = np.ascontiguousarray(bias.astype(np.float32).reshape(OUT, 1))
    xb = np.asarray(enc_x, dtype=np.float32).astype(bf16)                # [B, IN]
    return [
        {
            "xt": np.ascontiguousarray(xb[c * ROWS : (c + 1) * ROWS].T),
            "wt": wt,
            "bc": bc,
        }
        for c in range(N_CORES)
    ]


def gather_output(results) -> np.ndarray:
    out = np.empty((B, OUT), dtype=np.float32)
    for c in range(N_CORES):
        yt = np.asarray(results[c]["yt"])                                # [OUT, ROWS] bf16
        out[c * ROWS : (c + 1) * ROWS] = yt.T.astype(np.float32)
    return out


def kernel(enc_x: np.ndarray, weight: np.ndarray, bias: np.ndarray) -> np.ndarray:
    from concourse.bass_utils import run_bass_kernel_spmd

    in_maps = prep_in_maps(enc_x, weight, bias)
    res = run_bass_kernel_spmd(_get_nc(), in_maps, list(range(N_CORES)))
    return gather_output(res.results)


# revision 12
# speedup vs baseline: 1.0913x; 1.0913x over previous
"""Trainium2 Bass kernel for y = enc_x @ weight.T + bias.

Shapes (hardcoded): enc_x [524288, 128] f32, weight [128, 128] f32,
bias [128] f32 -> y [524288, 128] f32.

Strategy: data-parallel over 8 NeuronCores (65536 rows each), bf16 on the
wire. The tolerance gate (rel err < 2e-2) leaves ample room for bf16 I/O:
quantizing x and W to bf16 and the output y to bf16 gives ~4e-3 max rel
error while halving HBM traffic (the problem is memory-bound).

The host uploads x pre-transposed per core (x^T [128, 65536] bf16, feature
dim on partitions), so the device needs no on-chip transposes at all: the
tensor engine computes y^T = (W^T)^T-stationary @ x^T directly with N=512
streaming matmuls (W^T [128i, 128o] is the stationary operand, loaded from
SBUF each MM; x^T streams). PSUM fp32 accumulation; the vector engine adds
bias (per-partition scalar) while evicting PSUM -> SBUF with a bf16 cast.
The host transposes y^T back and upcasts to f32.

Input DMAs ride the SP HWDGE ring (nc.sync), output DMAs the ACT ring
(nc.scalar) so a store blocked on compute never head-of-line-blocks the
next prefetch.
"""

import numpy as np

B, IN, OUT = 524288, 128, 128
N_CORES = 8
ROWS = B // N_CORES            # 65536 rows per core
CHUNK = 4096                   # batch columns per SBUF tile (1 MiB bf16 DMA)
N_CHUNKS = ROWS // CHUNK       # 16
GROUP = 512                    # matmul N / one PSUM bank of f32
GROUPS = CHUNK // GROUP        # 8

_CACHE: dict = {}


def _build():
    import concourse.bacc as bacc
    import concourse.mybir as mybir
    import concourse.tile as tile
    from concourse.bass import ts

    nc = bacc.Bacc(
        "TRN2",
        target_bir_lowering=False,
        debug=False,
        enable_asserts=False,
        num_devices=N_CORES,
    )

    f32 = mybir.dt.float32
    bf16 = mybir.dt.bfloat16
    xt_d = nc.dram_tensor("xt", [IN, ROWS], bf16, kind="ExternalInput").ap()
    wt_d = nc.dram_tensor("wt", [IN, OUT], bf16, kind="ExternalInput").ap()
    bc_d = nc.dram_tensor("bc", [OUT, 1], f32, kind="ExternalInput").ap()
    yt_d = nc.dram_tensor("yt", [OUT, ROWS], bf16, kind="ExternalOutput").ap()

    with tile.TileContext(nc) as tc:
        with (
            tc.tile_pool(name="consts", bufs=1) as cpool,
            tc.tile_pool(name="xin", bufs=4) as xpool,
            tc.tile_pool(name="yout", bufs=8) as ypool,
            tc.tile_pool(name="ps", bufs=7, space="PSUM") as pspool,
            tc.tile_pool(name="warm", bufs=1, space="PSUM") as wpool,
        ):
            # Consts ride the ACT ring so the SP ring's first DMA is X[0].
            wt_sb = cpool.tile([IN, OUT], bf16)
            nc.scalar.dma_start(wt_sb[:], wt_d)
            b_sb = cpool.tile([OUT, 1], f32)
            nc.scalar.dma_start(b_sb[:], bc_d)

            # The PE HAM throttles the clock to 1.2 GHz after an idle
            # activity window; this burst of scratch matmuls runs while the
            # first input chunk is still in flight (the PE is idle anyway)
            # so the first real matmuls start at the warm 2.4 GHz clock.
            scratch = wpool.tile([OUT, OUT], f32)
            for _ in range(10):
                nc.tensor.matmul(
                    scratch[:], wt_sb[:], wt_sb[:], start=True, stop=True
                )

            for c in range(N_CHUNKS):
                X = xpool.tile([IN, CHUNK], bf16, tag="X")
                nc.sync.dma_start(X[:], xt_d[:, c * CHUNK : (c + 1) * CHUNK])
                YT = ypool.tile([OUT, CHUNK], bf16, tag="YT")
                for g in range(GROUPS):
                    ps = pspool.tile([OUT, GROUP], f32, tag="ps")
                    nc.tensor.matmul(
                        ps[:],
                        wt_sb[:],
                        X[:, ts(g, GROUP)],
                        start=True,
                        stop=True,
                    )
                    # PSUM eviction (+bias, f32->bf16 cast) alternates between
                    # DVE and ACT so neither engine's throughput binds.
                    if g % 2 == 0:
                        nc.vector.tensor_scalar_add(
                            YT[:, ts(g, GROUP)], ps[:], b_sb[:]
                        )
                    else:
                        nc.scalar.add(YT[:, ts(g, GROUP)], ps[:], b_sb[:])
                nc.scalar.dma_start(yt_d[:, c * CHUNK : (c + 1) * CHUNK], YT[:])

    nc.compile()
    return nc


def _get_nc():
    if "nc" not in _CACHE:
        _CACHE["nc"] = _build()
    return _CACHE["nc"]


def prep_in_maps(enc_x: np.ndarray, weight: np.ndarray, bias: np.ndarray):
    import ml_dtypes

    bf16 = ml_dtypes.bfloat16
    wt = np.ascontiguousarray(weight.astype(np.float32).T.astype(bf16))  # [IN, OUT]
    bc = np.ascontiguousarray(bias.astype(np.float32).reshape(OUT, 1))
    xb = np.asarray(enc_x, dtype=np.float32).astype(bf16)                # [B, IN]
    return [
        {
            "xt": np.ascontiguousarray(xb[c * ROWS : (c + 1) * ROWS].T),
            "wt": wt,
            "bc": bc,
        }
        for c in range(N_CORES)
    ]


def gather_output(results) -> np.ndarray:
    out = np.empty((B, OUT), dtype=np.float32)
    for c in range(N_CORES):
        yt = np.asarray(results[c]["yt"])                                # [OUT, ROWS] bf16
        out[c * ROWS : (c + 1) * ROWS] = yt.T.astype(np.float32)
    return out


def kernel(enc_x: np.ndarray, weight: np.ndarray, bias: np.ndarray) -> np.ndarray:
    from concourse.bass_utils import run_bass_kernel_spmd

    in_maps = prep_in_maps(enc_x, weight, bias)
    res = run_bass_kernel_spmd(_get_nc(), in_maps, list(range(N_CORES)))
    return gather_output(res.results)
